# revision 50
# baseline (speedup 1.0000x reference)
"""Trainium2 Bass kernel for nn_AVNNType1Conv2d (pair of 1x1 convs + elementwise
adjusted-mean derive), data-parallel over batch across 8 NeuronCores.

Reference computation (per batch b):
    act = x[b,:,:,:,0]                  # [C, H, W]
    car = x[b,:,:,:,1]
    act_out = relu(wx @ act + bx)       # 1x1 conv over channels
    rhs2    = act*car / (|act| + eps)   # elementwise derive (k=1 patches)
    car_out = wy @ rhs2 + by
    out[b]  = stack([act_out, car_out], -1)   # [O, H, W, 2]

Sharding: batch B=8 -> one batch per core, no cross-core communication.

The kernel is HBM-bound, so both the input image and the output are moved as
bf16 (the host converts; the 2e-2 rel-err budget dwarfs bf16's ~4e-3).  That
halves HBM traffic vs fp32: 32 MiB in + 32 MiB out per core over ~358 GB/s
per-core HBM bandwidth -> ~188 us floor (fp32 roofline was ~376 us; this
kernel measures ~210-217 us, engines at 67-85% under the DMA stream).

Per-core pipeline (x[b] is [C=128, H*W*2] contiguous bf16, act/car interleaved):
  DMA-in  (SP HWDGE ring):   xin[128, 2T] interleaved bf16 tile
  GPSIMD: num = act*car             (strided bf16 reads)
  DVE:    rhs2 = num/(|act|+eps) * (-c0^-2)  in ONE custom 8-stage op:
          t = max(act+eps, eps-act) = |act|+eps; BITWISE_NOT exponent-flip
          reciprocal seed + 1 Newton step (~1.7e-3 rel, fine under bf16);
          the Chebyshev constant -c0^2 is folded into wyT at setup.
  PE:     pa = wxT.T @ act, pb = wyT.T @ rhs2    (both bf16)
  ACT/DVE: out[...,0] = relu(pa+bx); out[...,1] = pb+by  (drain_pat splits
          the PSUM->SBUF drains ~5:1 ACT:DVE to balance engine load)
  DMA-out (ACT HWDGE ring):  contiguous interleaved bf16 tile
"""

import sys
import types

import numpy as np
from ml_dtypes import bfloat16

import concourse.bacc as bacc
import concourse.bass as bass
import concourse.dve_ops as dve_ops
import concourse.mybir as mybir
from concourse import bass_utils
from concourse.dve_spec import C0, C1, AluOp, Bin, Spec, Src0, Src1
from concourse.dve_spec import _has_src1
from concourse.dve_spec import lower as dve_lower
from concourse.dve_uop import DveOpSpec
from concourse.masks import make_identity
from concourse.tile import TileContext


def _ensure_axon_hooks_module():
    """bass_utils' axon trace path does `from antenv.axon_hooks import ...`;
    some images lack that submodule. Provide a no-op holder so tracing
    degrades gracefully instead of raising ImportError."""
    try:
        import antenv.axon_hooks  # noqa: F401
        return
    except ImportError:
        pass
    import antenv

    m = types.ModuleType("antenv.axon_hooks")
    m._hook = None
    m.get_axon_ntff_profile_hook = lambda: m._hook

    def _set(hook):
        m._hook = hook

    m.set_axon_ntff_profile_hook = _set
    antenv.axon_hooks = m
    sys.modules["antenv.axon_hooks"] = m


_ensure_axon_hooks_module()

B, C, H, W, O = 8, 128, 256, 256, 128
NPIX = H * W            # pixels per core (one batch per core)
EPS = 1e-6
F32 = mybir.dt.float32
BF16 = mybir.dt.bfloat16
ALU = mybir.AluOpType
AFT = mybir.ActivationFunctionType

# Fused DVE op: rhs2' = (NOT(t)*s1 + t*NOT(t)^2) * num with t = |a| + s0.
# This is the bitwise-NOT reciprocal seed + ONE Newton step (max rel err
# ~1.7e-3, fine under bf16), algebraically rearranged so only two scalar
# slots are needed (elementwise in1 forces the STT struct, which has no
# imm2 slot): with c0,c1 the Chebyshev pair, 1NR gives
#   y1 = c0*c1*nt - c0^2*t*nt^2 = B * (nt*(-c1/c0) + t*nt^2),  B = -c0^2
# The B factor is folded into the wy weights at setup.
_C0, _C1 = 0.23549792, 2.0017324
RECIP_S1 = float(np.float32(_C1 / _C0))          # exactly 8.5 in fp32
RECIP_B = float(np.float32(-(_C0 * _C0)))        # wyT pre-scale


def _ref_recip1nr_mul(in0, in1, s0, s1, imm2):
    a = in0.astype(np.float32)
    t = np.maximum(a + np.float32(s0), np.float32(s0) - a)   # |a| + eps
    nt = (~t.view(np.int32)).view(np.float32)
    return ((nt * (np.float32(s1) + t * nt)) * in1).astype(np.float32)


def _register_recip1nr_mul():
    """Register the fused op with the concourse custom-DVE registry (the
    documented extension point is appending to dve_ops.OPS; the repo is
    read-only here so do it at import time).

    8 stages: t = max(a+eps, eps-a) = |a|+eps; nt = NOT(t) (exponent-flip
    reciprocal seed); out = nt*(s1 + t*nt) * in1 — the 1-Newton-step
    reciprocal in a 2-constant form (the -c0^2 factor lives in wyT)."""
    name = "ANT_RECIP1NR_MUL"
    for o in dve_ops.OPS:
        if o.name == name:
            return o
    from concourse.dve_spec import maxx

    _t = maxx(Src0 + C0, C0 - Src0)
    _nt = Bin(AluOp.BITWISE_NOT, _t, _t)
    body = (_nt * (C1 + _t * _nt)) * Src1
    spec = Spec(body=body, reference=_ref_recip1nr_mul)
    row = dve_ops._CUSTOM_DVE_ROW_BASE + len(dve_ops.OPS)
    assert row < 0x20, "custom-DVE opcode rows exhausted"
    dve_ops._SUB_OPCODE_FOR_NAME[name] = row
    shas = {}
    for ver in ("v3", "v4"):
        try:
            uops = dve_lower(spec, ver=ver)
            shas[ver] = DveOpSpec(
                name=name, opcode=row, uops=uops, rd1_en=_has_src1(spec)
            ).sha(ver)
        except Exception:
            pass
    op = dve_ops.DveOp(name, spec, subdim=False, uops_sha=shas)
    dve_ops.OPS.append(op)
    dve_ops.CUSTOM_DVE_SPECS[name] = spec
    return op


RECIP1NR_MUL = _register_recip1nr_mul()


def _ref_recip1nr_mul_swap(in0, in1, s0, s1, imm2):
    return _ref_recip1nr_mul(in1, in0, s0, s1, imm2)


def _register_recip1nr_mul_swap():
    """Same op with operand roles swapped: in0 = num (contiguous fp32 on
    rd0), in1 = act (strided bf16 on rd1) — probes which read port handles
    the strided stream cheaper."""
    name = "ANT_RECIP1NR_MUL_SWAP"
    for o in dve_ops.OPS:
        if o.name == name:
            return o
    from concourse.dve_spec import maxx

    _t = maxx(Src1 + C0, C0 - Src1)
    _nt = Bin(AluOp.BITWISE_NOT, _t, _t)
    body = (_nt * (C1 + _t * _nt)) * Src0
    spec = Spec(body=body, reference=_ref_recip1nr_mul_swap)
    row = dve_ops._CUSTOM_DVE_ROW_BASE + len(dve_ops.OPS)
    assert row < 0x20, "custom-DVE opcode rows exhausted"
    dve_ops._SUB_OPCODE_FOR_NAME[name] = row
    shas = {}
    for ver in ("v3", "v4"):
        try:
            uops = dve_lower(spec, ver=ver)
            shas[ver] = DveOpSpec(
                name=name, opcode=row, uops=uops, rd1_en=_has_src1(spec)
            ).sha(ver)
        except Exception:
            pass
    op = dve_ops.DveOp(name, spec, subdim=False, uops_sha=shas)
    dve_ops.OPS.append(op)
    dve_ops.CUSTOM_DVE_SPECS[name] = spec
    return op


RECIP1NR_MUL_SWAP = _register_recip1nr_mul_swap()


def build_nc(npix=NPIX, tile_px=4096, mm_px=512, psum_px=1024, psum_bufs=2,
             in_bufs=3, out_bufs=2, mid_bufs=4,
             num_pat=("g",), psum_tag=None, num_bf16=False, swap_custom=False,
             planar=False, flush_tiles=0, abs_pat=("f",),
             drain_pat=("aa", "aa", "av"), last_split=1, head=None, tail=None,
             first_split=4, w_ring="sync", in_rings=("sync",),
             out_rings=("scalar",)):
    """Build the per-core Bass module. All 8 cores run the same program.

    DMA tiles are tile_px pixels (bf16: 8KB contiguous per partition at 2048).
    Compute runs over psum_px-pixel blocks with per-block intermediate tiles
    so everything fits in SBUF while input and output DMA streams stay fully
    decoupled (separate in/out SBUF tiles, separate HWDGE rings).
    """
    assert npix % tile_px == 0 and tile_px % psum_px == 0 and psum_px % mm_px == 0
    # graduated tile plan: optional small head/tail tiles so compute starts
    # sooner after the first load and the final drain tail is short
    head = list(head or [])
    tail = list(tail or [])
    mid_px = npix - sum(head) - sum(tail)
    assert mid_px % tile_px == 0
    tile_sizes = head + [tile_px] * (mid_px // tile_px) + list(tail)
    assert all(t % mm_px == 0 and t <= tile_px for t in tile_sizes)

    nc = bacc.Bacc("TRN2", target_bir_lowering=False, debug=False)
    x = nc.dram_tensor("x", [C, 2 * npix], BF16, kind="ExternalInput")
    wx = nc.dram_tensor("wx", [O, C], F32, kind="ExternalInput")
    bx = nc.dram_tensor("bx", [O, 1], F32, kind="ExternalInput")
    wy = nc.dram_tensor("wy", [O, C], F32, kind="ExternalInput")
    by = nc.dram_tensor("by", [O, 1], F32, kind="ExternalInput")
    out = nc.dram_tensor("out", [O, 2 * npix], BF16, kind="ExternalOutput")

    with TileContext(nc) as tc:
        with (
            tc.tile_pool(name="consts", bufs=1) as consts,
            tc.tile_pool(name="io_in", bufs=in_bufs) as io_in,
            tc.tile_pool(name="io_out", bufs=out_bufs) as io_out,
            tc.tile_pool(name="mid", bufs=mid_bufs) as mid,
            tc.tile_pool(name="psum", bufs=psum_bufs, space="PSUM") as psum,
        ):
            # ---- one-time setup: weights (transposed via PE), biases ----
            ident = consts.tile([128, 128], F32, tag="ident")
            make_identity(nc, ident[:])

            # weight/bias loads on the chosen ring; input (SP) ring's first
            # descriptors should be the first x tile
            w_eng = nc.sync if w_ring == "sync" else nc.scalar
            wxs = consts.tile([O, C], F32, tag="wxs")
            w_eng.dma_start(out=wxs[:], in_=wx[:])
            wys = consts.tile([O, C], F32, tag="wys")
            w_eng.dma_start(out=wys[:], in_=wy[:])
            bxs = consts.tile([O, 1], F32, tag="bxs")
            w_eng.dma_start(out=bxs[:], in_=bx[:])
            bys = consts.tile([O, 1], F32, tag="bys")
            w_eng.dma_start(out=bys[:], in_=by[:])

            # wxT[c, o] = wx[o, c]; PE transpose through PSUM (shares pa slot).
            wxT = consts.tile([C, O], BF16, tag="wxT")
            pt = psum.tile([C, O], F32, tag=psum_tag or "pa")
            nc.tensor.transpose(pt[:], wxs[:], ident[:])
            nc.vector.tensor_copy(out=wxT[:], in_=pt[:])

            # wyT scaled by RECIP_B (folds the 1NR-reciprocal constant)
            wyT = consts.tile([C, O], BF16, tag="wyT")
            pt2 = psum.tile([C, O], F32, tag=psum_tag or "pb")
            nc.tensor.transpose(pt2[:], wys[:], ident[:])
            nc.scalar.mul(wyT[:], pt2[:], RECIP_B)

            # ---- main loop over pixel tiles ----
            num_eng = [nc.gpsimd if e == "g" else nc.vector for e in num_pat]
            ring = {"sync": nc.sync, "scalar": nc.scalar, "vector": nc.vector,
                    "tensor": nc.tensor, "gpsimd": nc.gpsimd}
            in_rl = [ring[r] for r in in_rings]
            out_rl = [ring[r] for r in out_rings]
            n_seg = len(tile_sizes)
            pos = 0
            blk_i = 0
            for i, tpx in enumerate(tile_sizes):
                xin = io_in.tile([128, 2 * tile_px], BF16, tag="xin")
                xc0 = 2 * pos
                in_eng = in_rl[i % len(in_rl)]
                if planar:
                    # DRAM row = [act(npix) | car(npix)]; tile keeps act in
                    # [0:tile_px), car in [tile_px:2*tile_px) — all on-chip
                    # accesses contiguous, two DMA descriptors per tile.
                    if i == 0 and first_split > 1:
                        qw = tpx // first_split
                        for q in range(first_split):
                            in_eng.dma_start(
                                out=xin[:, q * qw : (q + 1) * qw],
                                in_=x[:, pos + q * qw : pos + (q + 1) * qw],
                            )
                            in_eng.dma_start(
                                out=xin[:, tile_px + q * qw : tile_px + (q + 1) * qw],
                                in_=x[:, npix + pos + q * qw : npix + pos + (q + 1) * qw],
                            )
                    else:
                        in_eng.dma_start(
                            out=xin[:, :tpx], in_=x[:, pos : pos + tpx]
                        )
                        in_eng.dma_start(
                            out=xin[:, tile_px : tile_px + tpx],
                            in_=x[:, npix + pos : npix + pos + tpx],
                        )
                elif i == 0 and first_split > 1:
                    # split the first load so compute starts earlier
                    qw = 2 * tpx // first_split
                    for q in range(first_split):
                        in_rl[q % len(in_rl)].dma_start(
                            out=xin[:, q * qw : (q + 1) * qw],
                            in_=x[:, xc0 + q * qw : xc0 + (q + 1) * qw],
                        )
                else:
                    in_eng.dma_start(
                        out=xin[:, : 2 * tpx], in_=x[:, xc0 : xc0 + 2 * tpx]
                    )
                x3 = xin[:, : 2 * tpx].rearrange("p (n two) -> p n two", two=2)
                outt = io_out.tile([128, 2 * tile_px], BF16, tag="outt")
                o3 = outt[:, : 2 * tpx].rearrange("p (n two) -> p n two", two=2)

                for base in range(0, tpx, psum_px):
                    npx = min(psum_px, tpx - base)
                    n_mm = npx // mm_px
                    hs = slice(base, base + npx)
                    if planar:
                        act = xin[:, base : base + npx]
                        car = xin[:, tile_px + base : tile_px + base + npx]
                    else:
                        act = x3[:, hs, 0]  # [128, npx], stride-2 bf16 views
                        car = x3[:, hs, 1]

                    # num = act*car (strided bf16 reads)
                    num = mid.tile(
                        [128, psum_px], BF16 if num_bf16 else F32, tag="num"
                    )
                    neng = num_eng[blk_i % len(num_eng)]
                    neng.tensor_tensor(
                        out=num[:, :npx], in0=act, in1=car, op=ALU.mult
                    )
                    # rhs2' = num / (|act|+eps) / RECIP_B, one fused DVE op.
                    # abs_pat 'a' blocks: ACT computes |act| first so the DVE
                    # op reads contiguous fp32 (cheaper than strided bf16);
                    # max(|a|+eps, eps-|a|) = |a|+eps, so the same op works.
                    if abs_pat[blk_i % len(abs_pat)] == "a":
                        mag = mid.tile([128, psum_px], F32, tag="mag")
                        nc.scalar.activation(
                            out=mag[:, :npx], in_=act, func=AFT.Abs
                        )
                        cin0 = mag[:, :npx]
                    else:
                        cin0 = act
                    rhs2 = mid.tile([128, psum_px], BF16, tag="rhs2")
                    if swap_custom:
                        nc.vector._custom_dve(
                            RECIP1NR_MUL_SWAP, out=rhs2[:, :npx],
                            in0=num[:, :npx], in1=act, s0=EPS, s1=RECIP_S1,
                        )
                    else:
                        nc.vector._custom_dve(
                            RECIP1NR_MUL, out=rhs2[:, :npx], in0=cin0,
                            in1=num[:, :npx], s0=EPS, s1=RECIP_S1,
                        )

                    pa = psum.tile([128, psum_px], F32, tag=psum_tag or "pa")
                    for j in range(n_mm):
                        nc.tensor.matmul(
                            pa[:, bass.ts(j, mm_px)],
                            wxT[:],
                            act[:, bass.ts(j, mm_px)],
                            start=True, stop=True,
                        )
                    pb = psum.tile([128, psum_px], F32, tag=psum_tag or "pb")
                    for j in range(n_mm):
                        nc.tensor.matmul(
                            pb[:, bass.ts(j, mm_px)],
                            wyT[:],
                            rhs2[:, bass.ts(j, mm_px)],
                            start=True, stop=True,
                        )
                    # drains: PSUM -> SBUF with bias (+relu for pa); both can
                    # run on either ACT ('a') or DVE ('v') — drain_pat cycles
                    # per block to balance the two engines' load.
                    if planar:
                        oa = outt[:, base : base + npx]
                        oc = outt[:, tile_px + base : tile_px + base + npx]
                    else:
                        oa = o3[:, hs, 0]
                        oc = o3[:, hs, 1]
                    pat = drain_pat[blk_i % len(drain_pat)]
                    if pat[0] == "a":
                        nc.scalar.activation(
                            out=oa, in_=pa[:, :npx], func=AFT.Relu,
                            bias=bxs[:]
                        )
                    else:
                        nc.vector.tensor_scalar(
                            out=oa, in0=pa[:, :npx], scalar1=bxs[:],
                            scalar2=0.0, op0=ALU.add, op1=ALU.max,
                        )
                    if pat[1] == "a":
                        nc.scalar.activation(
                            out=oc, in_=pb[:, :npx],
                            func=AFT.Identity, bias=bys[:],
                        )
                    else:
                        nc.vector.tensor_scalar(
                            out=oc, in0=pb[:, :npx], scalar1=bys[:],
                            scalar2=None, op0=ALU.add,
                        )
                    # final tiles: flush each drained block to DRAM right
                    # away, alternating rings (the in-ring is idle by then),
                    # so the tail drains at block rather than tile granularity
                    if not planar and i >= n_seg - flush_tiles:
                        feng = (in_rl + out_rl)[blk_i % 2]
                        feng.dma_start(
                            out=out[:, 2 * pos + 2 * base :
                                    2 * pos + 2 * (base + npx)],
                            in_=outt[:, 2 * base : 2 * (base + npx)],
                        )
                    blk_i += 1

                # output DMA on its own ring(s), decoupled from the input
                # ring(s). Last `last_split` tiles: drain across TWO rings so
                # the tail empties ~2x faster (input rings are idle by then).
                oc0 = 2 * pos
                out_eng = out_rl[i % len(out_rl)]
                if not planar and i >= n_seg - flush_tiles:
                    pass        # already flushed per block above
                elif planar:
                    split = i >= n_seg - last_split
                    eng2 = in_rl[0] if split else out_eng
                    out_eng.dma_start(
                        out=out[:, pos : pos + tpx], in_=outt[:, :tpx]
                    )
                    eng2.dma_start(
                        out=out[:, npix + pos : npix + pos + tpx],
                        in_=outt[:, tile_px : tile_px + tpx],
                    )
                elif i >= n_seg - last_split:
                    hw_ = tpx  # half of 2*tpx columns
                    out_eng.dma_start(
                        out=out[:, oc0 : oc0 + hw_], in_=outt[:, :hw_]
                    )
                    in_rl[0].dma_start(
                        out=out[:, oc0 + hw_ : oc0 + 2 * tpx],
                        in_=outt[:, hw_ : 2 * tpx],
                    )
                else:
                    out_eng.dma_start(
                        out=out[:, oc0 : oc0 + 2 * tpx], in_=outt[:, : 2 * tpx]
                    )
                pos += tpx
    nc.compile()
    return nc


_NC_CACHE = {}

# Set by the last kernel() call when BASS_TRACE=1: BassKernelResults with
# exec_time_ns from the NTFF profile of the slowest core.
LAST_RESULT = None

# Extra kwargs merged into the run_bass_kernel_spmd call (used by test.py to
# pass tmpdir/trace options; empty in production).
RUN_KWARGS = {}

# Build overrides for experiments from test.py.
BUILD_KWARGS = {}


def kernel(x, wx, bx, wy, by):
    global LAST_RESULT
    x = np.asarray(x, dtype=np.float32)
    wx = np.asarray(wx, dtype=np.float32)
    bx = np.asarray(bx, dtype=np.float32)
    wy = np.asarray(wy, dtype=np.float32)
    by = np.asarray(by, dtype=np.float32)
    assert x.shape == (B, C, H, W, 2)
    import json as _json

    key = _json.dumps(BUILD_KWARGS, sort_keys=True, default=str)
    if key not in _NC_CACHE:
        _NC_CACHE[key] = build_nc(**BUILD_KWARGS)
    nc = _NC_CACHE[key]

    # device moves bf16: convert once on host (256 MiB total)
    planar = BUILD_KWARGS.get("planar", False)
    if planar:
        xr = x.reshape(B, C, NPIX, 2)
        xb = np.empty((B, C, 2 * NPIX), dtype=bfloat16)
        xb[:, :, :NPIX] = xr[..., 0]
        xb[:, :, NPIX:] = xr[..., 1]
    else:
        xb = x.reshape(B, C, 2 * NPIX).astype(bfloat16)
    bx2 = np.ascontiguousarray(bx.reshape(O, 1), dtype=np.float32)
    by2 = np.ascontiguousarray(by.reshape(O, 1), dtype=np.float32)
    wxc = np.ascontiguousarray(wx, dtype=np.float32)
    wyc = np.ascontiguousarray(wy, dtype=np.float32)
    in_maps = [
        {"x": xb[b], "wx": wxc, "bx": bx2, "wy": wyc, "by": by2}
        for b in range(B)
    ]
    res = bass_utils.run_bass_kernel_spmd(
        nc, in_maps, core_ids=list(range(B)), **RUN_KWARGS
    )
    LAST_RESULT = res
    if planar:
        result = np.empty((B, O, H, W, 2), dtype=np.float32)
        rv = result.reshape(B, O, NPIX, 2)
        for b, r in enumerate(res.results):
            rv[b, :, :, 0] = r["out"][:, :NPIX]
            rv[b, :, :, 1] = r["out"][:, NPIX:]
        return result
    outs = [
        r["out"].astype(np.float32).reshape(O, H, W, 2) for r in res.results
    ]
    return np.stack(outs, axis=0)
